# revision 1
# baseline (speedup 1.0000x reference)
"""Grouped 2-layer MLP (ConvNN) Trainium2 kernel.

Math (per group g of SIZE=2048):
    h[b,g,:]   = LeakyReLU_0.2(W0[g] @ x[b] + b0[g])     (64 -> 64)
    out[b,g,:] = W1[g] @ h[b,g,:] + b1[g]                (64 -> 64)

Strategy:
  - Shard the group axis over 8 cores (256 groups/core), fully independent.
  - Host pre-transposes weights to [g, k, j] and x to xT so that both
    layers run on the PE with the contraction dim on partitions and the
    batch dim (1024) streaming as the moving operand. No on-chip
    transposes anywhere.
  - Groups are processed in pairs stacked on the 128 SBUF partitions;
    the two 64x64 matmuls of a pair land in opposite quadrants of the
    128x128 PE array and run concurrently (tile_position auto-derived
    from base partitions).
  - fp32r matmuls (1 PE cycle/row at N=512 vs 4 for fp32, ~11-bit
    mantissa). Walrus only allows ONE semaphore wait on an fp32r
    matmul (fused LDWEIGHTS struct), so deps are arranged to need at
    most one: weights+xT are staged via DMA then DVE-copied (with
    fp32r rounding) into resident tiles, making every mm0 dependency a
    DVE-sem tick; a single alternating PSUM tag makes every mm1
    dependency an ACT-sem tick.
  - h never touches HBM. Layer-0 bias+LeakyReLU is a single ScalarE
    activation (PSUM->SBUF, fp32r out); layer-1 bias add is a single
    VectorE tensor_scalar (PSUM->SBUF), splitting the two PSUM
    evacuation passes across ACT and DVE.
  - Output is written as [pair, 128, B] (contiguous 512KB DMA per pair)
    and un-transposed on the host.
"""

from contextlib import ExitStack

import numpy as np

import concourse.bass as bass
import concourse.mybir as mybir
import concourse.tile as tile
from concourse.bass_utils import run_bass_kernel_spmd

B = 1024
IN_DIM = 64
SIZE = 2048
D1 = 64
D2 = 64
NEG_SLOPE = 0.2
N_CORES = 8
GPC = SIZE // N_CORES  # 256 groups per core
NPAIR = GPC // 2  # 128 group-pairs per core
WB = 8  # group-pairs per weight DMA chunk
NCHUNK = NPAIR // WB

_NC_CACHE = None


def _build():
    global _NC_CACHE
    if _NC_CACHE is not None:
        return _NC_CACHE

    f32 = mybir.dt.float32
    f16 = mybir.dt.float16

    nc = bass.Bass()
    xT2 = nc.declare_dram_parameter("xT2", [128, B], f16, isOutput=False)
    w0t = nc.declare_dram_parameter("w0t", [128, NPAIR, D1], f16, isOutput=False)
    w1t = nc.declare_dram_parameter("w1t", [128, NPAIR, D2], f16, isOutput=False)
    b0p = nc.declare_dram_parameter("b0p", [128, NPAIR], f32, isOutput=False)
    b1p = nc.declare_dram_parameter("b1p", [128, NPAIR], f32, isOutput=False)
    out = nc.declare_dram_parameter("out", [NPAIR, 128, B], f32, isOutput=True)

    with ExitStack() as ctx:
        tc = ctx.enter_context(tile.TileContext(nc))
        singles = ctx.enter_context(tc.tile_pool(name="singles", bufs=1))
        stage = ctx.enter_context(tc.tile_pool(name="stage", bufs=3))
        hpool = ctx.enter_context(tc.tile_pool(name="hpool", bufs=3))
        opool = ctx.enter_context(tc.tile_pool(name="opool", bufs=3))
        pspool = ctx.enter_context(tc.tile_pool(name="psum", bufs=8, space="PSUM"))

        # biases, loaded once
        b0sb = singles.tile([128, NPAIR], f32)
        nc.sync.dma_start(out=b0sb, in_=b0p[:])
        b1sb = singles.tile([128, NPAIR], f32)
        nc.sync.dma_start(out=b1sb, in_=b1p[:])

        # xT resident, fp16 straight from HBM
        xt = singles.tile([128, B], f16)
        nc.sync.dma_start(out=xt, in_=xT2[:])

        # Weights as 128x128 block-diagonal fp32r tiles: one standard
        # K=128/M=128 matmul computes both groups of a pair (off-diagonal
        # zeros kill the cross terms; matmul time is N-driven so the
        # zeros cost nothing).  fp32r cannot col-tile (dst partition must
        # start at 0), which rules out 2x64-quadrant packing.
        # All weights land in SBUF via one contiguous 4MB DMA per layer;
        # ACT (w0) and DVE (w1) refresh only the diagonal blocks of
        # 2*WB ping-pong slots (with the fp32->fp32r rounding cast), so
        # block building for chunk k+1 overlaps PE compute on chunk k.
        w0s_all = singles.tile([128, NPAIR, D1], f16)
        nc.sync.dma_start(out=w0s_all, in_=w0t[:])
        w1s_all = singles.tile([128, NPAIR, D2], f16)
        nc.sync.dma_start(out=w1s_all, in_=w1t[:])

        w0d = singles.tile([128, 2 * WB, 128], f16)
        w1d = singles.tile([128, 2 * WB, 128], f16)
        nc.gpsimd.memset(w0d, 0.0)
        nc.gpsimd.memset(w1d, 0.0)

        for cki in range(NCHUNK):
            ck = bass.ts(cki, WB)
            half = slice((cki % 2) * WB, (cki % 2) * WB + WB)
            nc.scalar.copy(w0d[0:64, half, 0:64], w0s_all[0:64, ck, :])
            nc.scalar.copy(w0d[64:128, half, 64:128], w0s_all[64:128, ck, :])
            nc.scalar.copy(w1d[0:64, half, 0:64], w1s_all[0:64, ck, :])
            nc.scalar.copy(w1d[64:128, half, 64:128], w1s_all[64:128, ck, :])
            for q in range(WB):
                t = cki * WB + q
                qs = (cki % 2) * WB + q
                # half-pair (N=512) pipeline: 1-bank PSUM tiles so each
                # ACT/mm1/DVE stage starts as soon as half the batch is
                # ready -- keeps the dependency chain inside the 2-pair
                # PSUM lookahead and the PE dense (HAM stays warm).
                hps = [
                    pspool.tile([128, 512], f32, tag="ps", name=f"hps{t}_{i}")
                    for i in range(2)
                ]
                hsb = [
                    hpool.tile([128, 512], f16, tag=f"h{i}", name=f"hsb{t}_{i}")
                    for i in range(2)
                ]
                ops_ = [
                    pspool.tile([128, 512], f32, tag="ps", name=f"ops{t}_{i}")
                    for i in range(2)
                ]
                osb = opool.tile([128, B], f32, tag="o")
                for nb in range(2):
                    s = bass.ts(nb, 512)
                    nc.tensor.matmul(
                        hps[nb], w0d[:, qs, :], xt[:, s], start=True, stop=True
                    )
                    nc.scalar.activation(
                        out=hsb[nb],
                        in_=hps[nb],
                        func=mybir.ActivationFunctionType.Prelu,
                        bias=b0sb[:, t : t + 1],
                        scale=1.0,
                        alpha=NEG_SLOPE,
                    )
                    nc.tensor.matmul(
                        ops_[nb], w1d[:, qs, :], hsb[nb], start=True, stop=True
                    )
                    nc.vector.tensor_scalar_add(
                        osb[:, s], ops_[nb], b1sb[:, t : t + 1]
                    )
                nc.sync.dma_start(out=out[t], in_=osb)

    _split_multi_waits(nc)
    _NC_CACHE = nc
    return nc


def _split_multi_waits(nc):
    """Walrus in this toolchain allows at most ONE semaphore wait per
    instruction (and zero on the fused fp32 LDWEIGHTS struct).  Hoist all
    but the last wait of any multi-wait instruction onto same-engine NoOp
    carriers inserted directly before it — semantically identical (engine
    queues are in-order) and each carrier holds a single wait."""
    import bass_rust

    n = 0
    for f in nc.m.functions:
        for bb in f.blocks:
            out_insts = []
            changed = False
            for inst in bb.instructions:
                si = inst.sync_info
                waits = list(si.on_wait) if si is not None and si.on_wait else []
                if len(waits) > 1:
                    changed = True
                    for w in waits[:-1]:
                        nop = bass_rust.InstNoOp(
                            name=f"{inst.name}-sw{n}", engine=inst.engine
                        )
                        n += 1
                        nop.sync_info = mybir.SyncInfo(on_wait=[w], on_update=[])
                        out_insts.append(nop)
                    inst.sync_info = mybir.SyncInfo(
                        on_wait=[waits[-1]],
                        on_update=list(si.on_update) if si.on_update else [],
                    )
                out_insts.append(inst)
            if changed:
                bb.instructions = out_insts


def _prepare_in_maps(x, W0, b0, W1, b1):
    x = np.ascontiguousarray(np.asarray(x, dtype=np.float32))
    xT = x.T  # (64, 1024)
    xT2 = np.ascontiguousarray(
        np.concatenate([xT, xT], axis=0).astype(np.float16)
    )  # (128, 1024)
    in_maps = []
    for c in range(N_CORES):
        sl = slice(c * GPC, (c + 1) * GPC)
        W0c = np.asarray(W0[sl], dtype=np.float32)  # (256, 64, 64) [g, j, k]
        W1c = np.asarray(W1[sl], dtype=np.float32)
        # [g, k, j] then pair-stack on partitions: (NPAIR, 128, 64)
        w0t = np.ascontiguousarray(
            W0c.transpose(0, 2, 1).reshape(NPAIR, 128, D1).transpose(1, 0, 2)
        ).astype(np.float16)
        w1t = np.ascontiguousarray(
            W1c.transpose(0, 2, 1).reshape(NPAIR, 128, D2).transpose(1, 0, 2)
        ).astype(np.float16)
        b0p = np.ascontiguousarray(
            np.asarray(b0[sl], dtype=np.float32).reshape(NPAIR, 128).T
        )  # (128, NPAIR)
        b1p = np.ascontiguousarray(
            np.asarray(b1[sl], dtype=np.float32).reshape(NPAIR, 128).T
        )
        in_maps.append({"xT2": xT2, "w0t": w0t, "w1t": w1t, "b0p": b0p, "b1p": b1p})
    return in_maps


def _postprocess(results):
    outs = []
    for c in range(N_CORES):
        o = results[c]["out"]  # (NPAIR, 128, B) = [t, q*64+j, b]
        o = o.reshape(NPAIR, 2, 64, B).transpose(3, 0, 1, 2).reshape(B, GPC, D2)
        outs.append(o)
    return np.ascontiguousarray(np.concatenate(outs, axis=1))


def _run(inputs, trace=False):
    nc = _build()
    in_maps = _prepare_in_maps(**inputs)
    res = run_bass_kernel_spmd(
        nc, in_maps, core_ids=list(range(N_CORES)), trace=trace
    )
    return _postprocess(res.results), res


def kernel(x, W0, b0, W1, b1):
    out, _ = _run({"x": x, "W0": W0, "b0": b0, "W1": W1, "b1": b1})
    return out



# revision 2
# speedup vs baseline: 1.8067x; 1.8067x over previous
"""Grouped 2-layer MLP (ConvNN) Trainium2 kernel, v2.

Math (per group g of SIZE=2048):
    h[b,g,:]   = LeakyReLU_0.2(W0[g] @ x[b] + b0[g])     (64 -> 64)
    out[b,g,:] = W1[g] @ h[b,g,:] + b1[g]                (64 -> 64)

Strategy (v2 — pipelined, fp16 output):
  - Shard the group axis over 8 cores (256 groups/core = 128 pairs),
    fully independent.
  - Groups processed in pairs stacked on the 128 partitions.  Layer 0
    contracts over x (shared by all groups): stationary [K=64, M=128]
    compact weights, no zeros.  Layer 1 contracts per group: stationary
    is a host-built 128x128 block-diagonal tile (zeros cost nothing,
    matmul time is N-driven).
  - All weights land in SBUF via chunked DMAs straight into their final
    layout (no on-chip block building, no ACT/DVE copy work).
  - Per pair: h and out live in [128, 1024] fp32 PSUM tiles (2 banks
    each; 2 bufs per tag = all 8 banks).  One ACT Prelu (+b0, fp16 out)
    evacuates h; one DVE tensor_scalar (+b1, fp16 out) evacuates out.
    Large 1024-elem instructions amortize the fixed ACT/DVE overheads.
  - Explicit 2-stage software pipeline: iteration t issues layer-0 of
    pair t and layer-1 of pair t-2, so the PE never waits on the ACT
    evacuation of the same pair (the baseline's 1.3us/pair stall) and
    stays busy enough to hold the HAM clock gate at 2.4 GHz.
  - Output is written fp16 as [pair, 128, B] (256KB contiguous DMA per
    pair) and un-transposed/cast on the host.  Input DMAs ride SWDGE
    (gpsimd) so the HWDGE sync ring is dedicated to output stores.
"""

from contextlib import ExitStack

import numpy as np

import concourse.bass as bass
import concourse.mybir as mybir
import concourse.tile as tile
from concourse.bass_utils import run_bass_kernel_spmd

B = 1024
IN_DIM = 64
SIZE = 2048
D1 = 64
D2 = 64
NEG_SLOPE = 0.2
N_CORES = 8
GPC = SIZE // N_CORES  # 256 groups per core
NPAIR = GPC // 2  # 128 group-pairs per core
WCH = 4  # weight DMA chunks per layer
LAG = 2  # software-pipeline distance between layer 0 and layer 1

_NC_CACHE = None


def _build():
    global _NC_CACHE
    if _NC_CACHE is not None:
        return _NC_CACHE

    f32 = mybir.dt.float32
    f16 = mybir.dt.float16

    nc = bass.Bass()
    xt = nc.declare_dram_parameter("xt", [IN_DIM, B], f16, isOutput=False)
    w0c = nc.declare_dram_parameter("w0c", [IN_DIM, NPAIR, 128], f16, isOutput=False)
    w1d = nc.declare_dram_parameter("w1d", [128, NPAIR, 128], f16, isOutput=False)
    b0p = nc.declare_dram_parameter("b0p", [128, NPAIR], f32, isOutput=False)
    b1p = nc.declare_dram_parameter("b1p", [128, NPAIR], f32, isOutput=False)
    out = nc.declare_dram_parameter("out", [NPAIR, 128, B], f16, isOutput=True)

    with ExitStack() as ctx:
        tc = ctx.enter_context(tile.TileContext(nc))
        singles = ctx.enter_context(tc.tile_pool(name="singles", bufs=1))
        hpool = ctx.enter_context(tc.tile_pool(name="hpool", bufs=3))
        opool = ctx.enter_context(tc.tile_pool(name="opool", bufs=3))
        pspool = ctx.enter_context(tc.tile_pool(name="psum", bufs=2, space="PSUM"))

        b0sb = singles.tile([128, NPAIR], f32)
        nc.gpsimd.dma_start(out=b0sb, in_=b0p[:])
        b1sb = singles.tile([128, NPAIR], f32)
        nc.gpsimd.dma_start(out=b1sb, in_=b1p[:])
        xsb = singles.tile([IN_DIM, B], f16)
        nc.gpsimd.dma_start(out=xsb, in_=xt[:])

        # Weights streamed in chunks, interleaved so both layers' early
        # pairs arrive first; matmuls dep only on their covering chunk.
        w0sb = singles.tile([IN_DIM, NPAIR, 128], f16)
        w1sb = singles.tile([128, NPAIR, 128], f16)
        cw = NPAIR // WCH
        for c in range(WCH):
            ck = bass.ts(c, cw)
            nc.gpsimd.dma_start(out=w0sb[:, ck, :], in_=w0c[:, ck, :])
            nc.gpsimd.dma_start(out=w1sb[:, ck, :], in_=w1d[:, ck, :])

        hs_live = {}
        for t in range(NPAIR + LAG):
            if t < NPAIR:
                hp = pspool.tile([128, B], f32, tag="hps", name=f"hp{t}")
                nc.tensor.matmul(
                    hp[:, 0:512], w0sb[:, t, :], xsb[:, 0:512],
                    start=True, stop=True,
                )
                nc.tensor.matmul(
                    hp[:, 512:1024], w0sb[:, t, :], xsb[:, 512:1024],
                    start=True, stop=True,
                )
                hs = hpool.tile([128, B], f16, tag="h", name=f"hs{t}")
                nc.scalar.activation(
                    out=hs,
                    in_=hp,
                    func=mybir.ActivationFunctionType.Prelu,
                    bias=b0sb[:, t : t + 1],
                    scale=1.0,
                    alpha=NEG_SLOPE,
                )
                hs_live[t] = hs
            tp = t - LAG
            if tp >= 0:
                hs = hs_live.pop(tp)
                op = pspool.tile([128, B], f32, tag="ops", name=f"op{tp}")
                nc.tensor.matmul(
                    op[:, 0:512], w1sb[:, tp, :], hs[:, 0:512],
                    start=True, stop=True,
                )
                nc.tensor.matmul(
                    op[:, 512:1024], w1sb[:, tp, :], hs[:, 512:1024],
                    start=True, stop=True,
                )
                osb = opool.tile([128, B], f16, tag="o", name=f"os{tp}")
                nc.vector.tensor_scalar_add(osb, op, b1sb[:, tp : tp + 1])
                nc.sync.dma_start(out=out[tp], in_=osb)

    _split_multi_waits(nc)
    _NC_CACHE = nc
    return nc


def _split_multi_waits(nc):
    """Walrus in this toolchain allows at most ONE semaphore wait per
    instruction.  Hoist all but the last wait of any multi-wait
    instruction onto same-engine NoOp carriers inserted directly before
    it — semantically identical (engine queues are in-order) and each
    carrier holds a single wait."""
    import bass_rust

    n = 0
    for f in nc.m.functions:
        for bb in f.blocks:
            out_insts = []
            changed = False
            for inst in bb.instructions:
                si = inst.sync_info
                waits = list(si.on_wait) if si is not None and si.on_wait else []
                if len(waits) > 1:
                    changed = True
                    for w in waits[:-1]:
                        nop = bass_rust.InstNoOp(
                            name=f"{inst.name}-sw{n}", engine=inst.engine
                        )
                        n += 1
                        nop.sync_info = mybir.SyncInfo(on_wait=[w], on_update=[])
                        out_insts.append(nop)
                    inst.sync_info = mybir.SyncInfo(
                        on_wait=[waits[-1]],
                        on_update=list(si.on_update) if si.on_update else [],
                    )
                out_insts.append(inst)
            if changed:
                bb.instructions = out_insts
    return nc


def _prepare_in_maps(x, W0, b0, W1, b1):
    x = np.asarray(x, dtype=np.float32)
    xt = np.ascontiguousarray(x.T).astype(np.float16)  # (64, 1024)
    in_maps = []
    for c in range(N_CORES):
        sl = slice(c * GPC, (c + 1) * GPC)
        W0c = np.asarray(W0[sl], dtype=np.float32)  # (256, 64, 64) [g, j, k]
        W1c = np.asarray(W1[sl], dtype=np.float32)
        # Layer 0 compact: w0c[k, t, q*64+j] = W0[2t+q, j, k]
        w0ck = np.ascontiguousarray(
            W0c.reshape(NPAIR, 2, D1, IN_DIM)
            .transpose(3, 0, 1, 2)
            .reshape(IN_DIM, NPAIR, 128)
        ).astype(np.float16)
        # Layer 1 block-diagonal: w1d[(qr,k), t, (qc,j)] = [qr==qc]*W1[2t+qc, j, k]
        base = W1c.reshape(NPAIR, 2, D2, D1)  # [t, q, j, k]
        w1dk = np.zeros((2, D1, NPAIR, 2, D2), dtype=np.float32)
        for q in range(2):
            w1dk[q, :, :, q, :] = base[:, q, :, :].transpose(2, 0, 1)  # [k, t, j]
        w1dk = np.ascontiguousarray(w1dk.reshape(128, NPAIR, 128)).astype(np.float16)
        b0pc = np.ascontiguousarray(
            np.asarray(b0[sl], dtype=np.float32).reshape(NPAIR, 128).T
        )  # (128, NPAIR)
        b1pc = np.ascontiguousarray(
            np.asarray(b1[sl], dtype=np.float32).reshape(NPAIR, 128).T
        )
        in_maps.append(
            {"xt": xt, "w0c": w0ck, "w1d": w1dk, "b0p": b0pc, "b1p": b1pc}
        )
    return in_maps


def _postprocess(results):
    outs = []
    for c in range(N_CORES):
        o = results[c]["out"]  # (NPAIR, 128, B) fp16 = [t, q*64+j, b]
        o = (
            o.reshape(NPAIR, 2, 64, B)
            .transpose(3, 0, 1, 2)
            .reshape(B, GPC, D2)
            .astype(np.float32)
        )
        outs.append(o)
    return np.ascontiguousarray(np.concatenate(outs, axis=1))


def _run(inputs, trace=False):
    nc = _build()
    in_maps = _prepare_in_maps(**inputs)
    res = run_bass_kernel_spmd(
        nc, in_maps, core_ids=list(range(N_CORES)), trace=trace
    )
    return _postprocess(res.results), res


def kernel(x, W0, b0, W1, b1):
    out, _ = _run({"x": x, "W0": W0, "b0": b0, "W1": W1, "b1": b1})
    return out
